# revision 10
# baseline (speedup 1.0000x reference)
"""Trainium2 Bass kernel for nn_Attention: per-head QKV attention + out-proj.

Contract: kernel(**inputs) takes FULL unsharded inputs
  x [8, 1024, 768] f32, Wqkv [12, 768, 192] f32, bqkv [12, 192] f32,
  Wo [768, 768] f32, bo [768] f32
returns FULL output [8, 1024, 768] f32.

Strategy: pure data-parallel over batch (8 batches -> 8 NeuronCores), no
collectives.  Each core computes its batch end-to-end in bf16 matmuls.

v3 changes vs v2:
  - the two heads of a pair issue their scores matmuls back-to-back into
    DISJOINT PE row groups (K=64 each, tile_position (0,0)/(64,0)), so
    they stream concurrently: scores PE time halves (~20us).
  - softmax finalize rebuilt: broadcast the DENOMINATORS (not recips)
    with the ones65 matmul, one 128-lane reciprocal over the broadcast,
    then multiply.  Kills the per-pair rr memset/copy dance and the
    bf16 recast; finalize DVE drops ~4us/pair.
  - PSUM layout locked to exactly 8 banks: "sc" slot 2x[128,1024]
    (scores + all transients) and "pv" slot 2x[65,1024] (accumulators).
  - ~45 warmup matmuls on the ones65 tile fire as soon as its memset
    lands (~7.5us), flipping HAM to K=8/8 before the real prologue, and
    the DMA order puts wqk pair0 FIRST; bulk weights trigger from the
    (idle) gpsimd queue so the sync queue only carries the critical
    path.  First real matmul runs ~4us earlier and warm.
  - last pair's finalize multiplies straight out of PV psum (no u2
    staging) to shorten the output-projection tail.
"""

import math
import os
from collections import deque

import numpy as np
import ml_dtypes

import concourse.bass as bass
import concourse.tile as tile
from concourse import bacc, mybir
from concourse.bass_utils import run_bass_kernel_spmd
from concourse.tile_rust import add_dep_helper

B, S, D, H, HD = 8, 1024, 768, 12, 64
SCALE = 1.0 / math.sqrt(D)
FP = mybir.dt.float32
BF = mybir.dt.bfloat16
KC = D // 128   # 6 contraction chunks
SC = S // 128   # 8 seq chunks
NQ = S // 512   # 2 free-dim chunks of 512
NP = H // 2     # 6 head pairs
LAG = 3         # pv rides LAG slots behind scores

AluOp = mybir.AluOpType
ActFn = mybir.ActivationFunctionType

# Results of the last hardware run (for test harness introspection).
last_results = None


def _build_kernel_body(tc, out_d, xt_d, wqkp_d, wvp_d, wop_d, bqk_d, bo2_d):
    nc = tc.nc

    # Chain every TensorE instruction to the previous one with a no-sync
    # ordering edge so the Tile scheduler preserves the deliberate
    # scores/pv/filler interleave on the in-order PE stream.
    _pe_last = [None]

    def _chain(inst):
        if _pe_last[0] is not None:
            add_dep_helper(inst.ins, _pe_last[0].ins, sync=False,
                           reason="pe-order")
        _pe_last[0] = inst
        return inst

    def MM(*a, reuse_w=False, **k):
        inst = nc.tensor.matmul(*a, **k)
        if reuse_w:
            inst.ins.ldweights = False
        return _chain(inst)

    from contextlib import ExitStack

    with ExitStack() as ctx:
        wpool = ctx.enter_context(tc.tile_pool(name="weights", bufs=1))
        bigs = ctx.enter_context(tc.tile_pool(name="bigs", bufs=1))
        workp = ctx.enter_context(tc.tile_pool(name="work", bufs=1))
        outp = ctx.enter_context(tc.tile_pool(name="outstage", bufs=2))
        etp = ctx.enter_context(tc.tile_pool(name="et", bufs=4))
        scp = ctx.enter_context(tc.tile_pool(name="ps_t", bufs=2, space="PSUM"))
        pvp = ctx.enter_context(tc.tile_pool(name="ps_pv", bufs=2, space="PSUM"))

        # ---- persistent sbuf tensors ----
        xT = bigs.tile([128, KC, S], BF)
        wqk_sb = [wpool.tile([128, KC, 256], BF, name=f"wqk_{t}")
                  for t in range(NP)]
        wv_sb = wpool.tile([128, KC, D], BF)
        wo_sb = wpool.tile([128, KC, D], BF)
        bqk_sb = wpool.tile([128, 2 * KC], FP)
        bo_sb = wpool.tile([128, D], FP)
        qkT = bigs.tile([128, 2 * KC, S], BF)
        vaug = bigs.tile([128, SC, H * (HD + 1)], BF)
        vaug4 = vaug.rearrange("p s (h c) -> p s h c", c=HD + 1)
        outT = bigs.tile([128, KC, S], BF)
        # block-diagonal ones for the per-pair denominator broadcast: rows
        # 0:64 of the bcast output read partition 0 (head h0), rows 64:128
        # read partition 64 (head h1).  Engine writes must start at
        # partition 0/32/64/96, hence the K=65 shape with zero rows
        # 1..63 (rbf rows 1..63 stay zero so 0*0 contributes 0).
        ones65 = wpool.tile([65, 128], BF)
        rbf = bigs.tile([65, S], BF, name="rbf")
        wu = wpool.tile([128, 512], BF, name="wu")

        nc.gpsimd.memset(wu[:], 0.0)
        nc.gpsimd.memset(ones65[:], 0.0)
        nc.gpsimd.memset(ones65[0:1, 0:64], 1.0)
        nc.gpsimd.memset(ones65[64:65, 64:128], 1.0)
        nc.gpsimd.memset(rbf[:], 0.0)
        nc.gpsimd.memset(vaug4[:, :, :, HD:HD + 1], 1.0)

        # ---- DMAs, need-ordered ----
        # The sync queue carries only the critical path (wqk pair0 first,
        # then xT in kc order, then wv for the early v fillers); bulk
        # weights ride the otherwise-idle gpsimd queue.
        def dma_xt(kc, nsplit, eng):
            step = 128 // nsplit
            for i in range(nsplit):
                p0, p1 = i * step, (i + 1) * step
                eng.dma_start(xT[p0:p1, kc, :],
                              xt_d[kc * 128 + p0:kc * 128 + p1, :])

        def dma_wqk(t, nsplit, eng):
            w2 = wqkp_d[t * 128:(t + 1) * 128, :].rearrange(
                "p (kc f) -> p kc f", kc=KC)
            step = 128 // nsplit
            for i in range(nsplit):
                p0, p1 = i * step, (i + 1) * step
                eng.dma_start(wqk_sb[t][p0:p1, :, :], w2[p0:p1, :, :])

        # wqk pair0 kc0 (65KB, gates the very first matmul), then xT kc0,
        # then the rest of wqk0, then the xT kc stream the prologue
        # consumes in order; bulk weights follow strictly AFTER xT so they
        # never compete with it for HBM queue bandwidth.
        w0 = wqkp_d[0:128, :].rearrange("p (kc f) -> p kc f", kc=KC)
        nc.sync.dma_start(wqk_sb[0][:, 0, :], w0[:, 0, :])
        dma_xt(0, 4, nc.sync)
        nc.sync.dma_start(wqk_sb[0][:, 1:KC, :], w0[:, 1:KC, :])
        for kc in range(1, KC):
            dma_xt(kc, 2, nc.sync)
        bqk2 = bqk_d.rearrange("(p j) -> p j", p=128)
        for i in range(2):
            p0, p1 = i * 64, (i + 1) * 64
            nc.sync.dma_start(bqk_sb[p0:p1, :], bqk2[p0:p1, :])
        for half in range(2):
            p0, p1 = half * 64, (half + 1) * 64
            nc.sync.dma_start(wv_sb[p0:p1, :, :],
                              wvp_d[p0:p1, :].rearrange("p (kc f) -> p kc f",
                                                        kc=KC))
        for t in range(1, NP):
            dma_wqk(t, 2, nc.sync)
        nc.sync.dma_start(wo_sb[:],
                          wop_d.rearrange("p (kc f) -> p kc f", kc=KC))
        nc.sync.dma_start(
            bo_sb[:],
            bo2_d.rearrange("(a f) -> a f", a=1).partition_broadcast(128),
        )

        # ---- building blocks ----
        def qk_block(t, m, warmup=False):
            """Project one qk m-block (m: 0..5 = q of pair m, 6..11 = k of
            pair m-6) as a single piece: the psum tile's consumer (the
            bias-add evacuation) must follow immediately, because the "sc"
            slot rotation deadlocks if a release dep lands after a later
            allocation in the forced PE order."""
            pair = t
            qk = 0 if m < KC else 1
            col0 = qk * 128
            ps = scp.tile([128, S], FP, tag="sc", name=f"qk_{m}")
            if warmup:
                # HAM warmup: full-array (K=128, N=512) matmuls on the
                # memset wu tile (ready ~7.8us, long before any DMA input)
                # flip the PE clock gate to K=8/8 before the real
                # prologue.  Partial-array matmuls don't register enough
                # activity - measured: 4.8us of K=65/N=128 never flipped.
                # They write this same psum tile; the first real matmul's
                # start=True resets it, so no cleanup is needed.
                for i in range(12):
                    MM(ps[:, 0:512], wu[:, 0:128], wu[:],
                       start=True, stop=True, reuse_w=(i > 0))
            for kc in range(KC):
                lhsT = wqk_sb[pair][:, kc, col0:col0 + 128]
                for n in range(NQ):
                    MM(
                        ps[:, n * 512:(n + 1) * 512],
                        lhsT,
                        xT[:, kc, n * 512:(n + 1) * 512],
                        start=(kc == 0),
                        stop=(kc == KC - 1),
                        reuse_w=(n > 0),
                    )
            nc.vector.tensor_scalar_add(qkT[:, m, :], ps[:],
                                        bqk_sb[:, m:m + 1])

        def v_pieces(sc):
            """Yield 2 head-column pieces of one v chunk: heads 0-7
            (needed by pv from pair 0) then heads 8-11 (pair 4+)."""
            def piece(half):
                w = 512 if half == 0 else 256
                ps = scp.tile([128, S], FP, tag="sc", name=f"v_{sc}_{half}")
                for kc in range(KC):
                    MM(ps[:, 0:w],
                       xT[:, kc, sc * 128:(sc + 1) * 128],
                       wv_sb[:, kc, half * 512:half * 512 + w],
                       start=(kc == 0), stop=(kc == KC - 1))
                nc.vector.tensor_copy(
                    vaug4[:, sc, half * 8:half * 8 + w // HD, 0:HD],
                    ps[:, 0:w].rearrange("p (h c) -> p h c", c=HD),
                )

            yield lambda: piece(0)
            yield lambda: piece(1)

        def scores_pair(t, sk, et_pair):
            # Both heads' scores matmuls back-to-back: h0 occupies PE row
            # groups 0-1, h1 row groups 2-3 (K=64 each), so each n-block
            # pair streams CONCURRENTLY through separate xbuses.
            ps = [scp.tile([128, S], FP, tag="sc", name=f"sc_{t}_{sk}_{h}")
                  for h in range(2)]
            for n in range(NQ):
                for h01 in range(2):
                    lo, hi = h01 * 64, (h01 + 1) * 64
                    MM(
                        ps[h01][:, n * 512:(n + 1) * 512],
                        qkT[lo:hi, KC + t, sk * 128:(sk + 1) * 128],
                        qkT[lo:hi, t, n * 512:(n + 1) * 512],
                        start=True,
                        stop=True,
                        tile_position=(h01 * 64, 0),
                        reuse_w=(n > 0),
                    )
            for h01 in range(2):
                nc.scalar.activation(
                    et_pair[h01][:, sk, :], ps[h01][:], ActFn.Exp, scale=SCALE
                )

        def pv_chunk(t, sk, et_pair, pv_pair):
            for h01 in range(2):
                h = 2 * t + h01
                for n in range(NQ):
                    MM(
                        pv_pair[h01][:, n * 512:(n + 1) * 512],
                        vaug4[:, sk, h, :],
                        et_pair[h01][:, sk, n * 512:(n + 1) * 512],
                        start=(sk == 0),
                        stop=(sk == SC - 1),
                        reuse_w=(n > 0),
                    )

        def pv_finalize_a(t, pv_pair, last):
            # Stage the unnormalized sums to SBUF (frees the pv psum pair
            # for pair t+1) and pull the two denominator rows (the vaug
            # ones-column, partition 64) into rbf rows 0/64 as bf16.
            # For the LAST pair skip the u2 staging: the finalize multiply
            # reads pv psum directly, shortening the tail chain.
            u2 = None
            if not last:
                u2 = workp.tile([128, S], FP, tag="u2", name=f"u_{t}")
                nc.vector.tensor_copy(u2[0:HD, :], pv_pair[0][0:HD, :])
                nc.vector.tensor_copy(u2[64:64 + HD, :], pv_pair[1][0:HD, :])
            nc.vector.tensor_copy(rbf[0:1, :], pv_pair[0][HD:HD + 1, :])
            nc.vector.tensor_copy(rbf[64:65, :], pv_pair[1][HD:HD + 1, :])
            return u2

        def pv_finalize_b(t, u2, pv_pair):
            # Deferred a few slots so the fin_a DVE chain has completed.
            # bc = denominators broadcast across partitions (head h0 on
            # rows 0-63, h1 on 64-127); one 128-lane reciprocal; multiply.
            bc_ps = scp.tile([128, S], FP, tag="sc", name=f"bc_{t}")
            for n in range(NQ):
                MM(bc_ps[:, n * 512:(n + 1) * 512], ones65[:],
                   rbf[:, n * 512:(n + 1) * 512],
                   start=True, stop=True, reuse_w=(n > 0))
            rc = workp.tile([128, S], FP, tag="rc", name=f"rc_{t}")
            nc.vector.reciprocal_approx_fast(rc[:], bc_ps[:])
            for h01 in range(2):
                src = (u2[h01 * 64:h01 * 64 + HD, :] if u2 is not None
                       else pv_pair[h01][0:HD, :])
                nc.vector.tensor_tensor(
                    outT[h01 * 64:(h01 + 1) * 64, t, :],
                    src,
                    rc[h01 * 64:(h01 + 1) * 64, :],
                    op=AluOp.mult,
                )

        # ---- prologue: pair 0 q/k projections (stream behind xT DMA) ----
        qk_block(0, 0, warmup=True)
        qk_block(0, KC)

        # ---- main pipeline ----
        # Flat pipeline over 48 (pair, sk) chunks: pv(j-LAG) rides LAG
        # slots behind scores(j); v chunks (pair 0) and next-pair q/k
        # projections are emitted as <=6-MM filler pieces between the
        # scores/pv chunks of each slot.
        et_tiles = {}
        pv_tiles = {}
        fin_keep = {}
        filler = []  # (ready_slot, fn) in FIFO order per ready time

        def run_filler(j):
            for i, (rdy, fn) in enumerate(filler):
                if rdy <= j:
                    filler.pop(i)
                    fn()
                    return

        def emit_pv(j):
            t, sk = j // SC, j % SC
            if sk == 0:
                pv_tiles[t] = [
                    pvp.tile([HD + 1, S], FP, tag="pv", name=f"pv_{2 * t + i}")
                    for i in range(2)
                ]
            pv_chunk(t, sk, et_tiles[t], pv_tiles[t])
            if sk == SC - 1:
                last = (t == NP - 1)
                u2 = pv_finalize_a(t, pv_tiles[t], last)
                pvt = pv_tiles[t]
                cur = (t + 1) * SC + LAG - 2
                if last:
                    pv_finalize_b(t, u2, pvt)
                else:
                    filler.append((cur + 4,
                                   lambda t=t, u2=u2, pvt=pvt:
                                   pv_finalize_b(t, u2, pvt)))
                del pv_tiles[t], et_tiles[t]

        NCH = NP * SC
        for j in range(NCH):
            t, sk = j // SC, j % SC
            if sk == 0:
                et_tiles[t] = [
                    etp.tile([128, SC, S], BF, tag="et", name=f"et_{t}_{i}")
                    for i in range(2)
                ]
            if t == 0:
                # v heads 0-7 feed pv(0, sk) at slot sk+LAG; heads 8-11
                # are first read by pv at pair 4 - defer them a pair
                p1, p2 = v_pieces(sk)
                filler.append((j, p1))
                filler.append((j + 8, p2))
            if t + 1 < NP:
                if sk == 3:
                    filler.append((j, lambda t=t: qk_block(t + 1, t + 1)))
                elif sk == 4:
                    filler.append(
                        (j, lambda t=t: qk_block(t + 1, KC + t + 1)))
            # Slot order scores -> pv -> fillers: pv allocates no "sc"
            # tile, so it provides ~0.9us of allocation-free PE work that
            # covers the exp's hold on the psum buffer a filler (or the
            # next slot's scores) would otherwise stall on.
            scores_pair(t, sk, et_tiles[t])
            if j >= LAG:
                emit_pv(j - LAG)
            run_filler(j)
            run_filler(j)
        for j in range(NCH, NCH + LAG):
            emit_pv(j - LAG)
            while [f for f in filler if f[0] <= j]:
                run_filler(j)
        while filler:
            run_filler(10 ** 9)

        # ---- output projection ----
        for sc in range(SC):
            ps = scp.tile([128, S], FP, tag="sc", name=f"o_{sc}")
            for kc in range(KC):
                lhsT = outT[:, kc, sc * 128:(sc + 1) * 128]
                MM(ps[:, 0:512], lhsT, wo_sb[:, kc, 0:512],
                   start=(kc == 0), stop=(kc == KC - 1))
                MM(ps[:, 512:D], lhsT, wo_sb[:, kc, 512:D],
                   start=(kc == 0), stop=(kc == KC - 1), reuse_w=True)
            osb = outp.tile([128, D], FP, tag="osb", name=f"osb_{sc}")
            nc.vector.tensor_tensor(osb[:], ps[:, 0:D], bo_sb[:], op=AluOp.add)
            nc.sync.dma_start(out_d[sc * 128:(sc + 1) * 128, :], osb[:])


def build():
    """Build + compile the per-core Bass module. Returns the Bacc object."""
    nc = bacc.Bacc("TRN2", target_bir_lowering=False, debug=False, num_devices=B)
    xt_d = nc.dram_tensor("xt", [D, S], BF, kind="ExternalInput").ap()
    wqkp_d = nc.dram_tensor("wqkp", [NP * 128, KC * 256], BF,
                            kind="ExternalInput").ap()
    wvp_d = nc.dram_tensor("wvp", [128, KC * D], BF, kind="ExternalInput").ap()
    wop_d = nc.dram_tensor("wop", [128, KC * D], BF, kind="ExternalInput").ap()
    bqk_d = nc.dram_tensor("bqk", [2 * D], FP, kind="ExternalInput").ap()
    bo2_d = nc.dram_tensor("bo2", [D], FP, kind="ExternalInput").ap()
    out_d = nc.dram_tensor("out", [S, D], FP, kind="ExternalOutput").ap()
    with tile.TileContext(nc) as tc:
        _build_kernel_body(tc, out_d, xt_d, wqkp_d, wvp_d, wop_d, bqk_d, bo2_d)
    nc.compile()
    return nc


def prep_weights(Wqkv, bqkv, Wo, bo):
    """Host-side weight packing (numpy only)."""
    bf16 = ml_dtypes.bfloat16
    # Wqkv [H, D, 3*HD] -> Wq_all/Wk_all/Wv_all [D, H*HD]
    Wq = np.transpose(Wqkv[:, :, 0:HD], (1, 0, 2)).reshape(D, D)
    Wk = np.transpose(Wqkv[:, :, HD:2 * HD], (1, 0, 2)).reshape(D, D)
    Wv = np.transpose(Wqkv[:, :, 2 * HD:], (1, 0, 2)).reshape(D, D)
    # pair-major qk blocks: wqkp[t] = [128, KC, 256] with row p holding
    # W rows {kc*128+p} for all kc, cols = [q pair cols | k pair cols]
    wqkp = np.empty((NP, 128, KC, 256), dtype=bf16)
    for t in range(NP):
        blk = np.concatenate(
            [Wq[:, t * 128:(t + 1) * 128], Wk[:, t * 128:(t + 1) * 128]],
            axis=1,
        )  # [D, 256]
        wqkp[t] = blk.reshape(KC, 128, 256).transpose(1, 0, 2).astype(bf16)
    wqkp = wqkp.reshape(NP * 128, KC * 256)
    # per-partition-contiguous v / o weights: row p = [W[kc*128+p, :] for kc]
    wvp = Wv.reshape(KC, 128, D).transpose(1, 0, 2).reshape(128, KC * D)
    wop = Wo.reshape(KC, 128, D).transpose(1, 0, 2).reshape(128, KC * D)
    # biases: q then k, partition-major [p, j] with j = m-block id
    bq = bqkv[:, 0:HD].reshape(D)
    bk = bqkv[:, HD:2 * HD].reshape(D)
    bv = bqkv[:, 2 * HD:].reshape(D)
    bqk = np.concatenate([bq, bk]).reshape(2 * KC, 128).T  # [128, 12]
    bo2 = bo.astype(np.float64) + bv.astype(np.float64) @ Wo.astype(np.float64)
    return {
        "wqkp": np.ascontiguousarray(wqkp),
        "wvp": np.ascontiguousarray(wvp.astype(bf16)),
        "wop": np.ascontiguousarray(wop.astype(bf16)),
        "bqk": np.ascontiguousarray(bqk.reshape(2 * D).astype(np.float32)),
        "bo2": np.ascontiguousarray(bo2.astype(np.float32)),
    }


_nc_cache = None


def kernel(x, Wqkv, bqkv, Wo, bo):
    global _nc_cache, last_results
    if _nc_cache is None:
        _nc_cache = build()
    nc = _nc_cache
    w = prep_weights(np.asarray(Wqkv), np.asarray(bqkv), np.asarray(Wo),
                     np.asarray(bo))
    bf16 = ml_dtypes.bfloat16
    x = np.asarray(x, dtype=np.float32)
    in_maps = [
        {"xt": np.ascontiguousarray(x[i].T.astype(bf16)), **w}
        for i in range(B)
    ]
    res = run_bass_kernel_spmd(
        nc, in_maps, core_ids=list(range(B)),
        trace=bool(os.environ.get("KERNEL_TRACE")),
    )
    last_results = res
    out = np.stack([res.results[i]["out"] for i in range(B)], axis=0)
    return out.astype(np.float32)


# revision 17
# speedup vs baseline: 1.0050x; 1.0050x over previous
"""Trainium2 Bass kernel for nn_Attention: per-head QKV attention + out-proj.

Contract: kernel(**inputs) takes FULL unsharded inputs
  x [8, 1024, 768] f32, Wqkv [12, 768, 192] f32, bqkv [12, 192] f32,
  Wo [768, 768] f32, bo [768] f32
returns FULL output [8, 1024, 768] f32.

Strategy: pure data-parallel over batch (8 batches -> 8 NeuronCores), no
collectives.  Each core computes its batch end-to-end in bf16 matmuls.

v3 changes vs v2:
  - the two heads of a pair issue their scores matmuls back-to-back into
    DISJOINT PE row groups (K=64 each, tile_position (0,0)/(64,0)), so
    they stream concurrently: scores PE time halves (~20us).
  - softmax finalize rebuilt: broadcast the DENOMINATORS (not recips)
    with the ones65 matmul, one 128-lane reciprocal over the broadcast,
    then multiply.  Kills the per-pair rr memset/copy dance and the
    bf16 recast; finalize DVE drops ~4us/pair.
  - PSUM layout locked to exactly 8 banks: "sc" slot 2x[128,1024]
    (scores + all transients) and "pv" slot 2x[65,1024] (accumulators).
  - ~45 warmup matmuls on the ones65 tile fire as soon as its memset
    lands (~7.5us), flipping HAM to K=8/8 before the real prologue, and
    the DMA order puts wqk pair0 FIRST; bulk weights trigger from the
    (idle) gpsimd queue so the sync queue only carries the critical
    path.  First real matmul runs ~4us earlier and warm.
  - last pair's finalize multiplies straight out of PV psum (no u2
    staging) to shorten the output-projection tail.
"""

import math
import os
from collections import deque

import numpy as np
import ml_dtypes

import concourse.bass as bass
import concourse.tile as tile
from concourse import bacc, mybir
from concourse.bass_utils import run_bass_kernel_spmd
from concourse.tile_rust import add_dep_helper

B, S, D, H, HD = 8, 1024, 768, 12, 64
SCALE = 1.0 / math.sqrt(D)
FP = mybir.dt.float32
BF = mybir.dt.bfloat16
KC = D // 128   # 6 contraction chunks
SC = S // 128   # 8 seq chunks
NQ = S // 512   # 2 free-dim chunks of 512
NP = H // 2     # 6 head pairs
LAG = 3         # pv rides LAG slots behind scores

AluOp = mybir.AluOpType
ActFn = mybir.ActivationFunctionType

# Results of the last hardware run (for test harness introspection).
last_results = None


def _build_kernel_body(tc, out_d, xt_d, wqkp_d, wvp_d, wop_d, bqk_d, bo2_d):
    nc = tc.nc

    # Chain every TensorE instruction to the previous one with a no-sync
    # ordering edge so the Tile scheduler preserves the deliberate
    # scores/pv/filler interleave on the in-order PE stream.
    _pe_last = [None]

    def _chain(inst):
        if _pe_last[0] is not None:
            add_dep_helper(inst.ins, _pe_last[0].ins, sync=False,
                           reason="pe-order")
        _pe_last[0] = inst
        return inst

    def MM(*a, reuse_w=False, **k):
        inst = nc.tensor.matmul(*a, **k)
        if reuse_w:
            inst.ins.ldweights = False
        return _chain(inst)

    from contextlib import ExitStack

    with ExitStack() as ctx:
        wpool = ctx.enter_context(tc.tile_pool(name="weights", bufs=1))
        bigs = ctx.enter_context(tc.tile_pool(name="bigs", bufs=1))
        workp = ctx.enter_context(tc.tile_pool(name="work", bufs=1))
        outp = ctx.enter_context(tc.tile_pool(name="outstage", bufs=2))
        etp = ctx.enter_context(tc.tile_pool(name="et", bufs=4))
        scp = ctx.enter_context(tc.tile_pool(name="ps_t", bufs=2, space="PSUM"))
        pvp = ctx.enter_context(tc.tile_pool(name="ps_pv", bufs=2, space="PSUM"))

        # ---- persistent sbuf tensors ----
        xT = bigs.tile([128, KC, S], BF)
        wqk_sb = [wpool.tile([128, KC, 256], BF, name=f"wqk_{t}")
                  for t in range(NP)]
        wv_sb = wpool.tile([128, KC, D], BF)
        wo_sb = wpool.tile([128, KC, D], BF)
        bqk_sb = wpool.tile([128, 2 * KC], FP)
        bo_sb = wpool.tile([128, D], FP)
        qkT = bigs.tile([128, 2 * KC, S], BF)
        vaug = bigs.tile([128, SC, H * (HD + 1)], BF)
        vaug4 = vaug.rearrange("p s (h c) -> p s h c", c=HD + 1)
        outT = bigs.tile([128, KC, S], BF)
        # denominator-broadcast selector: row 64 set, rows 0..63 zero, so
        # ones65.T @ u2x[0:65] replicates u2x's row 64 (the softmax
        # denominator staged by fin_a) across all 64 output partitions of
        # a head's bcast matmul.  bf16 to match the u2 staging tiles
        # (fp32 matmuls are two-pass and can't reuse loaded weights).
        ones65 = wpool.tile([65, 64], BF)
        wu = wpool.tile([128, 512], BF, name="wu")

        nc.gpsimd.memset(wu[:], 0.0)
        nc.gpsimd.memset(ones65[:], 0.0)
        nc.gpsimd.memset(ones65[64:65, :], 1.0)
        nc.gpsimd.memset(vaug4[:, :, :, HD:HD + 1], 1.0)

        # ---- DMAs, need-ordered ----
        # The sync queue carries only the critical path (wqk pair0 first,
        # then xT in kc order, then wv for the early v fillers); bulk
        # weights ride the otherwise-idle gpsimd queue.
        def dma_xt(kc, nsplit, eng):
            step = 128 // nsplit
            for i in range(nsplit):
                p0, p1 = i * step, (i + 1) * step
                eng.dma_start(xT[p0:p1, kc, :],
                              xt_d[kc * 128 + p0:kc * 128 + p1, :])

        def dma_wqk(t, nsplit, eng):
            w2 = wqkp_d[t * 128:(t + 1) * 128, :].rearrange(
                "p (kc f) -> p kc f", kc=KC)
            step = 128 // nsplit
            for i in range(nsplit):
                p0, p1 = i * step, (i + 1) * step
                eng.dma_start(wqk_sb[t][p0:p1, :, :], w2[p0:p1, :, :])

        # wqk pair0 kc0 (65KB, gates the very first matmul), then xT kc0,
        # then the rest of wqk0, then the xT kc stream the prologue
        # consumes in order; bulk weights follow strictly AFTER xT so they
        # never compete with it for HBM queue bandwidth.
        w0 = wqkp_d[0:128, :].rearrange("p (kc f) -> p kc f", kc=KC)
        nc.sync.dma_start(wqk_sb[0][:, 0, :], w0[:, 0, :])
        dma_xt(0, 4, nc.sync)
        nc.sync.dma_start(wqk_sb[0][:, 1:KC, :], w0[:, 1:KC, :])
        for kc in range(1, KC):
            dma_xt(kc, 2, nc.sync)
        bqk2 = bqk_d.rearrange("(p j) -> p j", p=128)
        for i in range(2):
            p0, p1 = i * 64, (i + 1) * 64
            nc.sync.dma_start(bqk_sb[p0:p1, :], bqk2[p0:p1, :])
        for half in range(2):
            p0, p1 = half * 64, (half + 1) * 64
            nc.sync.dma_start(wv_sb[p0:p1, :, :],
                              wvp_d[p0:p1, :].rearrange("p (kc f) -> p kc f",
                                                        kc=KC))
        for t in range(1, NP):
            dma_wqk(t, 2, nc.sync)
        nc.sync.dma_start(wo_sb[:],
                          wop_d.rearrange("p (kc f) -> p kc f", kc=KC))
        nc.sync.dma_start(
            bo_sb[:],
            bo2_d.rearrange("(a f) -> a f", a=1).partition_broadcast(128),
        )

        # ---- building blocks ----
        def qk_block(t, m, warmup=False):
            """Project one qk m-block (m: 0..5 = q of pair m, 6..11 = k of
            pair m-6) as a single piece: the psum tile's consumer (the
            bias-add evacuation) must follow immediately, because the "sc"
            slot rotation deadlocks if a release dep lands after a later
            allocation in the forced PE order."""
            pair = t
            qk = 0 if m < KC else 1
            col0 = qk * 128
            ps = scp.tile([128, S], FP, tag="sc", name=f"qk_{m}")
            if warmup:
                # HAM warmup: full-array (K=128, N=512) matmuls on the
                # memset wu tile (ready ~7.8us, long before any DMA input)
                # flip the PE clock gate to K=8/8 before the real
                # prologue.  Partial-array matmuls don't register enough
                # activity - measured: 4.8us of K=65/N=128 never flipped.
                # They write this same psum tile; the first real matmul's
                # start=True resets it, so no cleanup is needed.
                for i in range(12):
                    MM(ps[:, 0:512], wu[:, 0:128], wu[:],
                       start=True, stop=True, reuse_w=(i > 0))
            for kc in range(KC):
                lhsT = wqk_sb[pair][:, kc, col0:col0 + 128]
                for n in range(NQ):
                    MM(
                        ps[:, n * 512:(n + 1) * 512],
                        lhsT,
                        xT[:, kc, n * 512:(n + 1) * 512],
                        start=(kc == 0),
                        stop=(kc == KC - 1),
                        reuse_w=(n > 0),
                    )
            nc.vector.tensor_scalar_add(qkT[:, m, :], ps[:],
                                        bqk_sb[:, m:m + 1])

        def v_pieces(sc):
            """Yield 2 head-column pieces of one v chunk: heads 0-7
            (needed by pv from pair 0) then heads 8-11 (pair 4+)."""
            def piece(half):
                w = 512 if half == 0 else 256
                ps = scp.tile([128, S], FP, tag="sc", name=f"v_{sc}_{half}")
                for kc in range(KC):
                    MM(ps[:, 0:w],
                       xT[:, kc, sc * 128:(sc + 1) * 128],
                       wv_sb[:, kc, half * 512:half * 512 + w],
                       start=(kc == 0), stop=(kc == KC - 1))
                nc.vector.tensor_copy(
                    vaug4[:, sc, half * 8:half * 8 + w // HD, 0:HD],
                    ps[:, 0:w].rearrange("p (h c) -> p h c", c=HD),
                )

            yield lambda: piece(0)
            yield lambda: piece(1)

        def scores_pair(t, sk, et_pair):
            # Both heads' scores matmuls back-to-back: h0 occupies PE row
            # groups 0-1, h1 row groups 2-3 (K=64 each), so each n-block
            # pair streams CONCURRENTLY through separate xbuses.
            ps = [scp.tile([128, S], FP, tag="sc", name=f"sc_{t}_{sk}_{h}")
                  for h in range(2)]
            for n in range(NQ):
                for h01 in range(2):
                    lo, hi = h01 * 64, (h01 + 1) * 64
                    MM(
                        ps[h01][:, n * 512:(n + 1) * 512],
                        qkT[lo:hi, KC + t, sk * 128:(sk + 1) * 128],
                        qkT[lo:hi, t, n * 512:(n + 1) * 512],
                        start=True,
                        stop=True,
                        tile_position=(h01 * 64, 0),
                        reuse_w=(n > 0),
                    )
            for h01 in range(2):
                nc.scalar.activation(
                    et_pair[h01][:, sk, :], ps[h01][:], ActFn.Exp, scale=SCALE
                )

        def pv_chunk(t, sk, et_pair, pv_pair):
            for h01 in range(2):
                h = 2 * t + h01
                for n in range(NQ):
                    MM(
                        pv_pair[h01][:, n * 512:(n + 1) * 512],
                        vaug4[:, sk, h, :],
                        et_pair[h01][:, sk, n * 512:(n + 1) * 512],
                        start=(sk == 0),
                        stop=(sk == SC - 1),
                        reuse_w=(n > 0),
                    )

        def pv_finalize_a(t, pv_pair):
            # Stage each head's full [65, S] accumulator (64 v-rows + the
            # denominator row) to SBUF in ONE copy per head: the pv psum
            # pair is released as early as possible for pair t+1.
            us = []
            for h01 in range(2):
                u = workp.tile([65, S], BF, tag=f"u2{h01}", name=f"u_{t}_{h01}")
                nc.vector.tensor_copy(u[:], pv_pair[h01][:])
                us.append(u)
            return us

        def pv_finalize_b(t, us):
            # Deferred a few slots so the fin_a DVE chain has completed.
            # Per head: broadcast u's denominator row across 64 partitions
            # with the ones65 selector matmul; one 128-lane reciprocal
            # covers both heads; multiply.
            bc_ps = scp.tile([128, S], FP, tag="sc", name=f"bc_{t}")
            for h01 in range(2):
                for n in range(NQ):
                    MM(bc_ps[h01 * 64:(h01 + 1) * 64, n * 512:(n + 1) * 512],
                       ones65[:],
                       us[h01][:, n * 512:(n + 1) * 512],
                       start=True, stop=True,
                       tile_position=(0, h01 * 64),
                       reuse_w=(n > 0))
            # reciprocal in place in PSUM; the multiplies then mix one SBUF
            # and one PSUM input (the equal-base-partition rule only
            # applies when BOTH inputs are SBUF).
            nc.vector.reciprocal_approx_fast(bc_ps[:], bc_ps[:])
            for h01 in range(2):
                nc.vector.tensor_tensor(
                    outT[h01 * 64:(h01 + 1) * 64, t, :],
                    us[h01][0:HD, :],
                    bc_ps[h01 * 64:(h01 + 1) * 64, :],
                    op=AluOp.mult,
                )

        # ---- prologue: pair 0 q/k projections (stream behind xT DMA) ----
        qk_block(0, 0, warmup=True)
        qk_block(0, KC)

        # ---- main pipeline ----
        # Flat pipeline over 48 (pair, sk) chunks: pv(j-LAG) rides LAG
        # slots behind scores(j); v chunks (pair 0) and next-pair q/k
        # projections are emitted as <=6-MM filler pieces between the
        # scores/pv chunks of each slot.
        et_tiles = {}
        pv_tiles = {}
        fin_keep = {}
        filler = []  # (ready_slot, fn) in FIFO order per ready time

        def run_filler(j):
            for i, (rdy, fn) in enumerate(filler):
                if rdy <= j:
                    filler.pop(i)
                    fn()
                    return

        def emit_pv(j):
            t, sk = j // SC, j % SC
            if sk == 0:
                pv_tiles[t] = [
                    pvp.tile([HD + 1, S], FP, tag="pv", name=f"pv_{2 * t + i}")
                    for i in range(2)
                ]
            pv_chunk(t, sk, et_tiles[t], pv_tiles[t])
            if sk == SC - 1:
                us = pv_finalize_a(t, pv_tiles[t])
                cur = (t + 1) * SC + LAG - 2
                if t == NP - 1:
                    pv_finalize_b(t, us)
                else:
                    filler.append((cur + 4,
                                   lambda t=t, us=us: pv_finalize_b(t, us)))
                del pv_tiles[t], et_tiles[t]

        NCH = NP * SC
        for j in range(NCH):
            t, sk = j // SC, j % SC
            if sk == 0:
                et_tiles[t] = [
                    etp.tile([128, SC, S], BF, tag="et", name=f"et_{t}_{i}")
                    for i in range(2)
                ]
            if t == 0:
                # v heads 0-7 feed pv(0, sk) at slot sk+LAG; heads 8-11
                # are first read by pv at pair 4 - defer them a pair
                p1, p2 = v_pieces(sk)
                filler.append((j, p1))
                filler.append((j + 8, p2))
            if t + 1 < NP:
                if sk == 3:
                    filler.append((j, lambda t=t: qk_block(t + 1, t + 1)))
                elif sk == 4:
                    filler.append(
                        (j, lambda t=t: qk_block(t + 1, KC + t + 1)))
            # Slot order scores -> pv -> fillers: pv allocates no "sc"
            # tile, so it provides ~0.9us of allocation-free PE work that
            # covers the exp's hold on the psum buffer a filler (or the
            # next slot's scores) would otherwise stall on.
            scores_pair(t, sk, et_tiles[t])
            if j >= LAG:
                emit_pv(j - LAG)
            run_filler(j)
            run_filler(j)
        for j in range(NCH, NCH + LAG):
            emit_pv(j - LAG)
            while [f for f in filler if f[0] <= j]:
                run_filler(j)
        while filler:
            run_filler(10 ** 9)

        # ---- output projection ----
        for sc in range(SC):
            ps = scp.tile([128, S], FP, tag="sc", name=f"o_{sc}")
            for kc in range(KC):
                lhsT = outT[:, kc, sc * 128:(sc + 1) * 128]
                MM(ps[:, 0:512], lhsT, wo_sb[:, kc, 0:512],
                   start=(kc == 0), stop=(kc == KC - 1))
                MM(ps[:, 512:D], lhsT, wo_sb[:, kc, 512:D],
                   start=(kc == 0), stop=(kc == KC - 1), reuse_w=True)
            osb = outp.tile([128, D], FP, tag="osb", name=f"osb_{sc}")
            nc.vector.tensor_tensor(osb[:], ps[:, 0:D], bo_sb[:], op=AluOp.add)
            nc.sync.dma_start(out_d[sc * 128:(sc + 1) * 128, :], osb[:])


def build():
    """Build + compile the per-core Bass module. Returns the Bacc object."""
    nc = bacc.Bacc("TRN2", target_bir_lowering=False, debug=False, num_devices=B)
    xt_d = nc.dram_tensor("xt", [D, S], BF, kind="ExternalInput").ap()
    wqkp_d = nc.dram_tensor("wqkp", [NP * 128, KC * 256], BF,
                            kind="ExternalInput").ap()
    wvp_d = nc.dram_tensor("wvp", [128, KC * D], BF, kind="ExternalInput").ap()
    wop_d = nc.dram_tensor("wop", [128, KC * D], BF, kind="ExternalInput").ap()
    bqk_d = nc.dram_tensor("bqk", [2 * D], FP, kind="ExternalInput").ap()
    bo2_d = nc.dram_tensor("bo2", [D], FP, kind="ExternalInput").ap()
    out_d = nc.dram_tensor("out", [S, D], FP, kind="ExternalOutput").ap()
    with tile.TileContext(nc) as tc:
        _build_kernel_body(tc, out_d, xt_d, wqkp_d, wvp_d, wop_d, bqk_d, bo2_d)
    nc.compile()
    return nc


def prep_weights(Wqkv, bqkv, Wo, bo):
    """Host-side weight packing (numpy only)."""
    bf16 = ml_dtypes.bfloat16
    # Wqkv [H, D, 3*HD] -> Wq_all/Wk_all/Wv_all [D, H*HD]
    Wq = np.transpose(Wqkv[:, :, 0:HD], (1, 0, 2)).reshape(D, D)
    Wk = np.transpose(Wqkv[:, :, HD:2 * HD], (1, 0, 2)).reshape(D, D)
    Wv = np.transpose(Wqkv[:, :, 2 * HD:], (1, 0, 2)).reshape(D, D)
    # pair-major qk blocks: wqkp[t] = [128, KC, 256] with row p holding
    # W rows {kc*128+p} for all kc, cols = [q pair cols | k pair cols]
    wqkp = np.empty((NP, 128, KC, 256), dtype=bf16)
    for t in range(NP):
        blk = np.concatenate(
            [Wq[:, t * 128:(t + 1) * 128], Wk[:, t * 128:(t + 1) * 128]],
            axis=1,
        )  # [D, 256]
        wqkp[t] = blk.reshape(KC, 128, 256).transpose(1, 0, 2).astype(bf16)
    wqkp = wqkp.reshape(NP * 128, KC * 256)
    # per-partition-contiguous v / o weights: row p = [W[kc*128+p, :] for kc]
    wvp = Wv.reshape(KC, 128, D).transpose(1, 0, 2).reshape(128, KC * D)
    wop = Wo.reshape(KC, 128, D).transpose(1, 0, 2).reshape(128, KC * D)
    # biases: q then k, partition-major [p, j] with j = m-block id
    bq = bqkv[:, 0:HD].reshape(D)
    bk = bqkv[:, HD:2 * HD].reshape(D)
    bv = bqkv[:, 2 * HD:].reshape(D)
    bqk = np.concatenate([bq, bk]).reshape(2 * KC, 128).T  # [128, 12]
    bo2 = bo.astype(np.float64) + bv.astype(np.float64) @ Wo.astype(np.float64)
    return {
        "wqkp": np.ascontiguousarray(wqkp),
        "wvp": np.ascontiguousarray(wvp.astype(bf16)),
        "wop": np.ascontiguousarray(wop.astype(bf16)),
        "bqk": np.ascontiguousarray(bqk.reshape(2 * D).astype(np.float32)),
        "bo2": np.ascontiguousarray(bo2.astype(np.float32)),
    }


_nc_cache = None


def kernel(x, Wqkv, bqkv, Wo, bo):
    global _nc_cache, last_results
    if _nc_cache is None:
        _nc_cache = build()
    nc = _nc_cache
    w = prep_weights(np.asarray(Wqkv), np.asarray(bqkv), np.asarray(Wo),
                     np.asarray(bo))
    bf16 = ml_dtypes.bfloat16
    x = np.asarray(x, dtype=np.float32)
    in_maps = [
        {"xt": np.ascontiguousarray(x[i].T.astype(bf16)), **w}
        for i in range(B)
    ]
    res = run_bass_kernel_spmd(
        nc, in_maps, core_ids=list(range(B)),
        trace=bool(os.environ.get("KERNEL_TRACE")),
    )
    last_results = res
    out = np.stack([res.results[i]["out"] for i in range(B)], axis=0)
    return out.astype(np.float32)
